# revision 1
# baseline (speedup 1.0000x reference)
"""CAM (channel attention) module kernel for Trainium2, 8-core data-parallel.

Reference computation (per batch b, channel c):
    v = x[b,c]                         # (P=3, HW=4096)
    energy = v @ v.T                   # (3,3) Gram matrix
    en = rowmax(energy) - energy
    att = softmax(en, axis=-1)
    out = att @ v                      # (3, 4096)
    y[b,c] = gamma * out + x[b,c]

Sharding: batch dim (B=8) across the 8 NeuronCores; no cross-core comms.
Per core: channels on SBUF partitions (2 groups of 128), free dim = P*HW.
  - Gram diagonal:  ScalarE Square activation with accum_out (free-axis sum)
  - Gram off-diag:  VectorE scalar_tensor_tensor with accum_out (fused
                    elementwise product + free-axis sum in one op)
  - softmax:        tiny [128, 3x3] ops on VectorE/ScalarE
  - mix (att @ v):  identity+gamma folded into coeffs C = gamma*att + I, then
                    per row i: ScalarE premult, two VectorE scalar_tensor_tensor
Measured steady-state: ~79 us/core vs the 70.4 us HBM roofline
(25.2 MB/core @ 358 GB/s); VectorE-bound (18 fused ops x ~4.4 us).
"""

import numpy as np

import concourse.bacc as bacc
import concourse.mybir as mybir
import concourse.tile as tile
from concourse.bass_utils import run_bass_kernel_spmd

B, C, P, H, W = 8, 256, 3, 64, 64
HW = H * W
N_CORES = 8
PARTS = 128

F32 = mybir.dt.float32
Alu = mybir.AluOpType
Act = mybir.ActivationFunctionType


def build_nc(C_=C, HW_=HW, repeat=1, split_load=True):
    """Build the per-core Bass program. Each core sees x:(C_,P,HW_), gamma:(1,1).

    repeat>1 re-runs the whole computation (same I/O, idempotent) that many
    times in one program — used by test.py to time the kernel by slope.
    split_load: 3 per-path load DMAs (measured faster than one big DMA).
    """
    assert C_ % PARTS == 0
    n_groups = C_ // PARTS

    nc = bacc.Bacc("TRN2", target_bir_lowering=False, debug=False)

    x_d = nc.dram_tensor("x", [C_, P, HW_], F32, kind="ExternalInput")
    g_d = nc.dram_tensor("gamma", [1, 1], F32, kind="ExternalInput")
    y_d = nc.dram_tensor("y", [C_, P, HW_], F32, kind="ExternalOutput")

    with tile.TileContext(nc) as tc:
        with (
            tc.tile_pool(name="consts", bufs=1) as consts,
            tc.tile_pool(name="vpool", bufs=2) as vpool,
            tc.tile_pool(name="scratch", bufs=1) as scratch,
            tc.tile_pool(name="tpool", bufs=3) as tpool,
            tc.tile_pool(name="smalls", bufs=2) as smalls,
        ):
            # --- constants (once) ---
            gsb = consts.tile([1, 1], F32)
            nc.sync.dma_start(gsb[:], g_d[:])
            gamma_bc = consts.tile([PARTS, 1], F32)
            nc.gpsimd.partition_broadcast(gamma_bc[:], gsb[:])

            ident = consts.tile([PARTS, 9], F32)
            nc.vector.memset(ident[:], 0.0)
            for i in range(P):
                nc.vector.memset(ident[:, 4 * i : 4 * i + 1], 1.0)

            for g in range(n_groups * repeat):
                g = g % n_groups
                cs = slice(g * PARTS, (g + 1) * PARTS)

                # --- load group: 3 path-split DMAs so compute starts early ---
                v = vpool.tile([PARTS, P, HW_], F32)
                if split_load:
                    for i in range(P):
                        nc.sync.dma_start(v[:, i, :], x_d[cs, i, :])
                else:
                    nc.sync.dma_start(v[:], x_d[cs, :, :])

                # --- phase 1: per-channel 3x3 Gram matrix over HW ---
                E = smalls.tile([PARTS, 9], F32)
                # per-engine scratch tags (write-only garbage): one slot per
                # engine keeps ACT and DVE from serializing on shared slots
                for i in range(P):  # diagonal terms on ScalarE
                    scr = scratch.tile([PARTS, HW_], F32, tag="scr_act", bufs=1)
                    nc.scalar.activation(
                        scr[:], v[:, i, :], Act.Square,
                        accum_out=E[:, 4 * i : 4 * i + 1],
                    )
                for i, j, col in ((0, 1, 1), (1, 2, 5), (0, 2, 2)):  # cross on VectorE
                    # NOTE: tensor_tensor_reduce wedges the exec unit on this
                    # runtime (NRT_EXEC_UNIT_UNRECOVERABLE); scalar_tensor_tensor
                    # with accum_out is the same fused mult+reduce via the
                    # standard TensorScalarPtr opcode and works.
                    scr = scratch.tile([PARTS, HW_], F32, tag="scr_dve", bufs=1)
                    nc.vector.scalar_tensor_tensor(
                        scr[:], v[:, i, :], 1.0, v[:, j, :],
                        op0=Alu.bypass, op1=Alu.mult,
                        accum_out=E[:, col : col + 1],
                    )
                # mirror symmetric entries: (1,0)<-(0,1), (2,1)<-(1,2), (2,0)<-(0,2)
                # on ScalarE (Copy) to keep VectorE's instruction count down
                for src, dst in ((1, 3), (5, 7), (2, 6)):
                    nc.scalar.copy(E[:, dst : dst + 1], E[:, src : src + 1])

                # --- softmax over rows of the 3x3, coeffs = gamma*att + I ---
                E3 = E.rearrange("p (i j) -> p i j", j=P)
                M = smalls.tile([PARTS, P, 1], F32)
                # reference computes softmax(rowmax - E); softmax is shift
                # invariant, so use (rowmin - E) instead: exponents stay <= 0
                # (numerically stable without a second max pass).
                nc.vector.tensor_reduce(M[:], E3, axis=mybir.AxisListType.X, op=Alu.min)
                # EX[i,j] = exp(rowmin_i - E[i,j]) via per-row ScalarE Exp with
                # scale=-1, bias=rowmin_i (per-partition AP) — no DVE subtract
                EX = smalls.tile([PARTS, P, P], F32)
                for i in range(P):
                    nc.scalar.activation(
                        EX[:, i, :], E3[:, i, :], Act.Exp,
                        scale=-1.0, bias=M[:, i, 0:1],
                    )
                S = smalls.tile([PARTS, P, 1], F32)
                nc.vector.tensor_reduce(S[:], EX[:], axis=mybir.AxisListType.X, op=Alu.add)
                R = smalls.tile([PARTS, P, 1], F32)
                nc.vector.reciprocal(R[:], S[:])
                A = smalls.tile([PARTS, P, P], F32)
                nc.vector.tensor_mul(A[:], EX[:], R[:].broadcast_to([PARTS, P, P]))
                Cf = smalls.tile([PARTS, 9], F32)
                nc.vector.scalar_tensor_tensor(
                    Cf[:].rearrange("p (i j) -> p i j", j=P), A[:], gamma_bc[:, 0:1],
                    ident[:].rearrange("p (i j) -> p i j", j=P),
                    op0=Alu.mult, op1=Alu.add,
                )

                # --- phase 2: y_i = Cf[i,0]*v0 + Cf[i,1]*v1 + Cf[i,2]*v2 ---
                # one 3-slot tag per row: ACT premult (row i+2), DVE mix
                # (row i+1), out-DMA (row i) pipeline without extra SBUF
                for i in range(P):
                    t = tpool.tile([PARTS, HW_], F32, tag="t", bufs=3)
                    # t = Cf[i,1] * v1   (ScalarE)
                    nc.scalar.activation(
                        t[:], v[:, 1, :], Act.Copy,
                        scale=Cf[:, 3 * i + 1 : 3 * i + 2],
                    )
                    # t += Cf[i,0] * v0  (VectorE fused, in-place add)
                    nc.vector.scalar_tensor_tensor(
                        t[:], v[:, 0, :], Cf[:, 3 * i : 3 * i + 1], t[:],
                        op0=Alu.mult, op1=Alu.add,
                    )
                    # t += Cf[i,2] * v2  (VectorE fused, in-place)
                    nc.vector.scalar_tensor_tensor(
                        t[:], v[:, 2, :], Cf[:, 3 * i + 2 : 3 * i + 3], t[:],
                        op0=Alu.mult, op1=Alu.add,
                    )
                    nc.sync.dma_start(y_d[cs, i, :], t[:])

    nc.compile()
    return nc


_NC_CACHE = {}


def _get_nc(C_=C, HW_=HW):
    key = (C_, HW_)
    if key not in _NC_CACHE:
        _NC_CACHE[key] = build_nc(C_, HW_)
    return _NC_CACHE[key]


def run_full(x: np.ndarray, gamma: np.ndarray, **runner_kwargs):
    """Run on all 8 cores; returns the raw BassKernelResults."""
    x = np.asarray(x, dtype=np.float32)
    gamma = np.asarray(gamma, dtype=np.float32)
    assert x.shape == (B, C, P, H, W), x.shape

    nc = _get_nc()
    in_maps = [
        {
            "x": np.ascontiguousarray(x[k]).reshape(C, P, HW),
            "gamma": gamma.reshape(1, 1),
        }
        for k in range(N_CORES)
    ]
    return run_bass_kernel_spmd(
        nc, in_maps, core_ids=list(range(N_CORES)), **runner_kwargs
    )


def kernel(x: np.ndarray, gamma: np.ndarray) -> np.ndarray:
    res = run_full(x, gamma)
    y = np.stack([res.results[k]["y"] for k in range(N_CORES)])
    return y.reshape(B, C, P, H, W)

